# revision 2
# baseline (speedup 1.0000x reference)
"""Block-circulant linear layer (y = x @ W^T + bias, W built from 64x64
circulant blocks) on 8 Trainium2 NeuronCores.

Math: per output block j, input block i: y[t,j] = sum_i circ(c[j,i]) @ x[t,i].
Via the convolution theorem this is, for each rfft bin k:
    Yhat[t,j,k] = sum_i Chat[j,i,k] * Xhat[t,i,k]   (complex)
i.e. 33 independent complex [64 x 64] matmuls over the block index, batched
over tokens. The host does the cheap O(T*F*logB) DFTs + layout packing; the
device does the dominant compute — the per-frequency complex matmuls — packed
as 32 real [128x128] @ [128x512] matmuls per core (data-parallel over tokens).

Real/complex packing (per frequency k, contraction over rows r):
    rhs rows r:   [Xr_i (64) ; Xi_i (64)],  cols = tokens
    lhsT[i,    j] =  Cr[j,i]    lhsT[i,    64+j] = Ci[j,i]
    lhsT[64+i, j] = -Ci[j,i]    lhsT[64+i, 64+j] = Cr[j,i]
    out rows:     [Yr_j (64) ; Yi_j (64)]
Bins k=0 and k=32 are purely real (real input DFT), so they share one tile
(kt=0) with a block-diagonal lhsT; kt=1..31 carry bin k = kt.
"""

import numpy as np

_B = 64          # circulant block size
_NBLK = 64       # input/output blocks (4096/64)
_NK = 33         # rfft bins of a 64-point real signal
_NKT = 32        # packed frequency tiles (k0+k32 share tile 0)
_NCORES = 8
_T = 4096        # tokens = 2*2048
_TCORE = _T // _NCORES
_F = 4096

_CACHE = {}


def _build_cmat(c):
    """c: [J=64, I=64, B=64] float32 -> packed lhsT matrix [128, NKT*128]."""
    fc = np.fft.rfft(np.asarray(c, np.float32), axis=-1)  # [J, I, 33] complex64
    Cr, Ci = fc.real, fc.imag
    cm = np.zeros((_NKT, 128, 128), np.float32)  # [kt, row, col]
    cm[0, 0:64, 0:64] = Cr[:, :, 0].T
    cm[0, 64:128, 64:128] = Cr[:, :, 32].T
    for k in range(1, 32):
        cm[k, 0:64, 0:64] = Cr[:, :, k].T
        cm[k, 64:128, 0:64] = -Ci[:, :, k].T
        cm[k, 0:64, 64:128] = Ci[:, :, k].T
        cm[k, 64:128, 64:128] = Cr[:, :, k].T
    # device layout: [128 partitions, kt*128 + col]
    return np.ascontiguousarray(cm.transpose(1, 0, 2)).reshape(128, _NKT * 128)


def _build_xk(x):
    """x: [2, 2048, 4096] float32 -> packed rhs [NKT, 128, T]."""
    xb = np.asarray(x, np.float32).reshape(_T, _NBLK, _B)
    fx = np.fft.rfft(xb, axis=-1)            # [T, I, 33] complex64
    R = fx.real.transpose(2, 1, 0)           # [33, I, T]
    Im = fx.imag.transpose(2, 1, 0)
    XKf = np.empty((_NKT, 128, _T), np.float32)
    XKf[0, 0:64] = R[0]
    XKf[0, 64:128] = R[32]
    XKf[1:32, 0:64] = R[1:32]
    XKf[1:32, 64:128] = Im[1:32]
    return XKf


def _unpack_y(YKf, bias):
    """YKf: [NKT, 128, T] device output -> y [2, 2048, 4096] float32."""
    re = np.zeros((_NK, _NBLK, _T), np.float32)
    im = np.zeros((_NK, _NBLK, _T), np.float32)
    re[0] = YKf[0, 0:64]
    re[32] = YKf[0, 64:128]
    re[1:32] = YKf[1:32, 0:64]
    im[1:32] = YKf[1:32, 64:128]
    Yf = (re + 1j * im).transpose(2, 1, 0)   # [T, J, 33]
    yb = np.fft.irfft(Yf, n=_B, axis=-1).astype(np.float32)  # [T, J, B]
    y = yb.reshape(_T, _F) + np.asarray(bias, np.float32)
    return np.ascontiguousarray(y.reshape(2, _T // 2, _F))


def _build_device():
    import concourse.bacc as bacc
    import concourse.mybir as mybir
    import concourse.tile as tile

    f32 = mybir.dt.float32
    nc = bacc.Bacc("TRN2", target_bir_lowering=False, debug=False)
    xk = nc.dram_tensor("xk", [_NKT, 128, _TCORE], f32, kind="ExternalInput")
    cm = nc.dram_tensor("cm", [128, _NKT * 128], f32, kind="ExternalInput")
    yk = nc.dram_tensor("yk", [_NKT, 128, _TCORE], f32, kind="ExternalOutput")

    NCH = 4                      # weight tiles per DMA chunk group
    KT_CH = _NKT // NCH          # 8 kt per chunk

    with tile.TileContext(nc) as tc:
        with (
            tc.tile_pool(name="cpool", bufs=1) as cpool,
            tc.tile_pool(name="xpool", bufs=6) as xpool,
            tc.tile_pool(name="ypool", bufs=6) as ypool,
            tc.tile_pool(name="pp", bufs=4, space="PSUM") as pp,
        ):
            cts = []
            for g in range(NCH):
                ct = cpool.tile([128, KT_CH * 128], f32, tag=f"cw{g}")
                nc.sync.dma_start(
                    out=ct[:],
                    in_=cm[:, g * KT_CH * 128:(g + 1) * KT_CH * 128],
                )
                cts.append(ct)
            for kt in range(_NKT):
                xt = xpool.tile([128, _TCORE], f32)
                nc.sync.dma_start(out=xt[:], in_=xk[kt])
                ps = pp.tile([128, _TCORE], f32)
                g, r = divmod(kt, KT_CH)
                nc.tensor.matmul(
                    ps[:],
                    lhsT=cts[g][:, r * 128:(r + 1) * 128],
                    rhs=xt[:],
                    start=True,
                    stop=True,
                )
                yt = ypool.tile([128, _TCORE], f32)
                nc.vector.tensor_copy(yt[:], ps[:])
                nc.sync.dma_start(out=yk[kt], in_=yt[:])
    nc.compile()
    return nc


def _execute(in_maps, **kwargs):
    from concourse.bass_utils import run_bass_kernel_spmd

    if "nc" not in _CACHE:
        _CACHE["nc"] = _build_device()
    return run_bass_kernel_spmd(
        _CACHE["nc"], in_maps, core_ids=list(range(_NCORES)), **kwargs
    )


def _make_in_maps(x, c):
    XKf = _build_xk(x)
    cmd = _build_cmat(c)
    return [
        {
            "xk": np.ascontiguousarray(XKf[:, :, m * _TCORE:(m + 1) * _TCORE]),
            "cm": cmd,
        }
        for m in range(_NCORES)
    ]


def kernel(x, c, bias, **_kwargs):
    in_maps = _make_in_maps(x, c)
    bkr = _execute(in_maps)
    YKf = np.concatenate(
        [np.asarray(r["yk"]) for r in bkr.results], axis=2
    )  # [NKT, 128, T]
    return _unpack_y(YKf, bias)


# revision 6
# speedup vs baseline: 1.2735x; 1.2735x over previous
"""Block-circulant linear layer (y = x @ W^T + bias, W built from 64x64
circulant blocks) on 8 Trainium2 NeuronCores.

Math: per output block j, input block i: y[t,j] = sum_i circ(c[j,i]) @ x[t,i].
Via the convolution theorem this is, for each rfft bin k:
    Yhat[t,j,k] = sum_i Chat[j,i,k] * Xhat[t,i,k]   (complex)
i.e. 33 independent complex [64 x 64] matmuls over the block index, batched
over tokens. The host does the cheap O(T*F*logB) DFTs + layout packing; the
device does the dominant compute — the per-frequency complex matmuls — packed
as 32 real [128x128] @ [128x512] matmuls per core (data-parallel over tokens).

Real/complex packing (per frequency k, contraction over rows r):
    rhs rows r:   [Xr_i (64) ; Xi_i (64)],  cols = tokens
    lhsT[i,    j] =  Cr[j,i]    lhsT[i,    64+j] = Ci[j,i]
    lhsT[64+i, j] = -Ci[j,i]    lhsT[64+i, 64+j] = Cr[j,i]
    out rows:     [Yr_j (64) ; Yi_j (64)]
Bins k=0 and k=32 are purely real (real input DFT), so they share one tile
(kt=0) with a block-diagonal lhsT; kt=1..31 carry bin k = kt.
"""

import numpy as np

_B = 64          # circulant block size
_NBLK = 64       # input/output blocks (4096/64)
_NK = 33         # rfft bins of a 64-point real signal
_NKT = 32        # packed frequency tiles (k0+k32 share tile 0)
_NCORES = 8
_T = 4096        # tokens = 2*2048
_TCORE = _T // _NCORES
_F = 4096

_CACHE = {}


# 4 k-tiles ride in each DMA transfer (1 MB): free-dim layout (kt%KTB, t)
_KTB = 4
_NG = _NKT // _KTB   # 8 DMA groups


def _build_cmat(c):
    """c: [J=64, I=64, B=64] float32 -> packed lhsT matrix [128, NKT*128]."""
    fc = np.fft.rfft(np.asarray(c, np.float32), axis=-1)  # [J, I, 33] complex64
    Cr, Ci = fc.real, fc.imag
    cm = np.zeros((_NKT, 128, 128), np.float32)  # [kt, row, col]
    cm[0, 0:64, 0:64] = Cr[:, :, 0].T
    cm[0, 64:128, 64:128] = Cr[:, :, 32].T
    for k in range(1, 32):
        cm[k, 0:64, 0:64] = Cr[:, :, k].T
        cm[k, 64:128, 0:64] = -Ci[:, :, k].T
        cm[k, 0:64, 64:128] = Ci[:, :, k].T
        cm[k, 64:128, 64:128] = Cr[:, :, k].T
    # device layout: [128 partitions, kt*128 + col]
    return np.ascontiguousarray(cm.transpose(1, 0, 2)).reshape(128, _NKT * 128)


def _build_xk(x):
    """x: [2, 2048, 4096] float32 -> packed rhs [NKT, 128, T]."""
    xb = np.asarray(x, np.float32).reshape(_T, _NBLK, _B)
    fx = np.fft.rfft(xb, axis=-1)            # [T, I, 33] complex64
    R = fx.real.transpose(2, 1, 0)           # [33, I, T]
    Im = fx.imag.transpose(2, 1, 0)
    XKf = np.empty((_NKT, 128, _T), np.float32)
    XKf[0, 0:64] = R[0]
    XKf[0, 64:128] = R[32]
    XKf[1:32, 0:64] = R[1:32]
    XKf[1:32, 64:128] = Im[1:32]
    return XKf


def _unpack_y(YKf, bias):
    """YKf: [NKT, 128, T] device output -> y [2, 2048, 4096] float32."""
    re = np.zeros((_NK, _NBLK, _T), np.float32)
    im = np.zeros((_NK, _NBLK, _T), np.float32)
    re[0] = YKf[0, 0:64]
    re[32] = YKf[0, 64:128]
    re[1:32] = YKf[1:32, 0:64]
    im[1:32] = YKf[1:32, 64:128]
    Yf = (re + 1j * im).transpose(2, 1, 0)   # [T, J, 33]
    yb = np.fft.irfft(Yf, n=_B, axis=-1).astype(np.float32)  # [T, J, B]
    y = yb.reshape(_T, _F) + np.asarray(bias, np.float32)
    return np.ascontiguousarray(y.reshape(2, _T // 2, _F))


def _build_device():
    import concourse.bacc as bacc
    import concourse.mybir as mybir
    import concourse.tile as tile

    f32 = mybir.dt.float32
    # float32r: same fp32 bits, but the PE streams 1 column/cycle instead of
    # fp32's 4 (cost: reduced multiply precision; accumulation stays fp32).
    mmdt = mybir.dt.float32r
    nc = bacc.Bacc("TRN2", target_bir_lowering=False, debug=False)
    xk = nc.dram_tensor("xk", [_NG, 128, _KTB * _TCORE], mmdt, kind="ExternalInput")
    cm = nc.dram_tensor("cm", [128, _NKT * 128], mmdt, kind="ExternalInput")
    yk = nc.dram_tensor("yk", [_NG, 128, _KTB * _TCORE], f32, kind="ExternalOutput")

    with tile.TileContext(nc) as tc:
        with (
            tc.tile_pool(name="cpool", bufs=1) as cpool,
            tc.tile_pool(name="xpool", bufs=3) as xpool,
            tc.tile_pool(name="ypool", bufs=3) as ypool,
            tc.tile_pool(name="pp", bufs=6, space="PSUM") as pp,
        ):
            cts = []
            for g2 in range(2):
                ct = cpool.tile([128, (_NKT // 2) * 128], mmdt, tag=f"cw{g2}")
                nc.sync.dma_start(
                    out=ct[:],
                    in_=cm[:, g2 * (_NKT // 2) * 128:(g2 + 1) * (_NKT // 2) * 128],
                )
                cts.append(ct)
            for g in range(_NG):
                xt = xpool.tile([128, _KTB * _TCORE], mmdt)
                nc.sync.dma_start(out=xt[:], in_=xk[g])
                yt = ypool.tile([128, _KTB * _TCORE], f32)
                for j in range(_KTB):
                    kt = g * _KTB + j
                    g2, r = divmod(kt, _NKT // 2)
                    ps = pp.tile([128, _TCORE], f32)
                    nc.tensor.matmul(
                        ps[:],
                        lhsT=cts[g2][:, r * 128:(r + 1) * 128],
                        rhs=xt[:, j * _TCORE:(j + 1) * _TCORE],
                        start=True,
                        stop=True,
                    )
                    nc.vector.tensor_copy(yt[:, j * _TCORE:(j + 1) * _TCORE], ps[:])
                # stores ride the ACT HWDGE ring, loads the SP ring
                nc.scalar.dma_start(out=yk[g], in_=yt[:])
    nc.compile()
    return nc


def _execute(in_maps, **kwargs):
    from concourse.bass_utils import run_bass_kernel_spmd

    if "nc" not in _CACHE:
        _CACHE["nc"] = _build_device()
    return run_bass_kernel_spmd(
        _CACHE["nc"], in_maps, core_ids=list(range(_NCORES)), **kwargs
    )


def _make_in_maps(x, c):
    XKf = _build_xk(x)
    cmd = _build_cmat(c)
    maps = []
    for m in range(_NCORES):
        xkm = XKf[:, :, m * _TCORE:(m + 1) * _TCORE]  # [NKT, 128, TCORE]
        xkm = (
            xkm.reshape(_NG, _KTB, 128, _TCORE)
            .transpose(0, 2, 1, 3)
            .reshape(_NG, 128, _KTB * _TCORE)
        )
        maps.append({"xk": np.ascontiguousarray(xkm), "cm": cmd})
    return maps


def _gather_yk(results):
    """Per-core yk [NG, 128, KTB*TCORE] -> full [NKT, 128, T]."""
    per_core = []
    for r in results:
        ykm = np.asarray(r["yk"]).reshape(_NG, 128, _KTB, _TCORE)
        per_core.append(ykm.transpose(0, 2, 1, 3).reshape(_NKT, 128, _TCORE))
    return np.concatenate(per_core, axis=2)


def kernel(x, c, bias, **_kwargs):
    in_maps = _make_in_maps(x, c)
    bkr = _execute(in_maps)
    return _unpack_y(_gather_yk(bkr.results), bias)
